# revision 26
# baseline (speedup 1.0000x reference)
"""DeepseekV2 (MLA) attention prefill kernel for 8 Trainium2 NeuronCores.

Sharding: tensor-parallel over the 16 heads (2 heads/core: wq/wkv_b output
cols + wo input rows sharded), token-sliced low-rank KV stage (each core
computes 512 tokens of the compressed latent, AllGather redistributes it),
host sums the 8 partial output projections.

Layout strategy (everything "transposed", feature-on-partition):
  - hsT [H, B*S] streamed from DRAM; q/k projections produce qT/knopeT
    [d, tok] directly, V is produced in [tok, d] — the exact operand
    layouts attention needs, so the kernel contains zero transposes.
  - scores are computed as scoresT [k_tok, q_tok]; softmax skips the max
    subtraction (|score*scale| <= ~8 for this model, exp is safe), the
    denominator comes from a ones-matmul that also broadcasts it across
    partitions, so normalization is a plain elementwise multiply.
"""

import sys

if "/opt/trn_rl_repo" not in sys.path:
    sys.path.insert(0, "/opt/trn_rl_repo")

import numpy as np
import ml_dtypes

import concourse.bass as bass
import concourse.mybir as mybir
import concourse.tile as tile
from concourse import bacc
from concourse import bass_isa
from concourse.bass_utils import run_bass_kernel_spmd

F32 = mybir.dt.float32
F32R = mybir.dt.float32r
BF16 = mybir.dt.bfloat16
I32 = mybir.dt.int32
AF = mybir.ActivationFunctionType

B, S, H = 2, 2048, 2048
NH = 16
D_NOPE = 128
D_ROPE = 64
D_V = 128
LORA = 512
ROPE_BASE = 1e6
SCALING = float(D_V) ** -0.5
EPS = 1e-6

N_CORES = 8
NTOK = B * S                 # 4096 flattened tokens
TSLICE = NTOK // N_CORES     # 512 tokens/core for the latent stage
HPC = NH // N_CORES          # 2 heads per core

PI = float(np.pi)
TWO_PI = 2.0 * PI


def _cody_waite_consts():
    """Split 2*pi into c1+c2+c3 with c1/c2 exact under k<=~512 multiply."""
    def trunc12(x):
        u = np.float32(x).view(np.uint32) & np.uint32(0xFFFFF000)
        return float(u.view(np.float32))
    c1 = trunc12(TWO_PI)
    c2 = trunc12(TWO_PI - c1)
    c3 = float(np.float32(TWO_PI - c1 - c2))
    return c1, c2, c3


def _build():
    nc = bacc.Bacc("TRN2", target_bir_lowering=False, debug=False,
                   num_devices=N_CORES)

    # ---- per-core I/O ----
    hsT = nc.declare_dram_parameter("hsT", [H, NTOK], BF16, isOutput=False)
    hslT = nc.declare_dram_parameter("hslT", [H, TSLICE], F32, isOutput=False)
    pos = nc.declare_dram_parameter("pos", [1, TSLICE], I32, isOutput=False)
    wq_sh = nc.declare_dram_parameter("wq_sh", [H, HPC * D_NOPE], BF16, isOutput=False)
    wkv_a = nc.declare_dram_parameter("wkv_a", [H, LORA + D_ROPE], F32, isOutput=False)
    lnw = nc.declare_dram_parameter("lnw", [128, LORA // 128], F32, isOutput=False)
    # [LORA, 512]: cols = [h0_k(128) | h1_k(128) | h0_v(128) | h1_v(128)]
    wkv_b_sh = nc.declare_dram_parameter("wkv_b_sh", [LORA, HPC * (D_NOPE + D_V)],
                                         BF16, isOutput=False)
    wo_sh = nc.declare_dram_parameter("wo_sh", [HPC * D_V, H], F32, isOutput=False)

    out_p = nc.declare_dram_parameter("out_p", [NTOK, H], F32, isOutput=True)
    kvfT = nc.declare_dram_parameter("kvfT", [LORA + D_ROPE, TSLICE], F32,
                                     isOutput=True)

    # ---- constants (baked into the NEFF) ----
    ones_np = np.ones((128, 128), np.float32)
    invf_np = (1.0 / (ROPE_BASE ** (np.arange(32, dtype=np.float64) / 32.0))
               ).astype(np.float32).reshape(1, 32)
    # mask_j[p, f] = 1 if f >= p + 128*j (valid, k<=q) else 0 ; j = 0..3
    f_idx = np.arange(512)[None, :]
    p_idx = np.arange(128)[:, None]
    masks_np = np.concatenate(
        [(f_idx >= p_idx + 128 * j).astype(np.float32) for j in range(4)], axis=1)
    ones_dram = nc.inline_tensor(ones_np, name="ones_c")
    invf_dram = nc.inline_tensor(invf_np, name="invf_c")
    masks_dram = nc.inline_tensor(masks_np, name="masks_c")

    c1, c2, c3 = _cody_waite_consts()

    KC = H // 128        # 16 contraction chunks over hidden dim
    LC = LORA // 128     # 4 contraction chunks over lora dim
    SEQ_TT = S // 512    # 4 512-token tiles per sequence
    SEQ_KT = S // 128    # 16 128-token key tiles per sequence

    with tile.TileContext(nc) as tc:
        ctx_pools = []

        pconst = tc.alloc_tile_pool(name="pconst", bufs=1)
        pw = tc.alloc_tile_pool(name="pw", bufs=1)
        pdram = tc.alloc_tile_pool(name="pdram", bufs=1, space="DRAM")
        ctx_pools += [pconst, pw, pdram]

        # constants to SBUF
        ones_sb = pconst.tile([128, 128], F32R, tag="ones")
        nc.gpsimd.dma_start(ones_sb[:], ones_dram[:].bitcast(F32R))
        invf_sb = pconst.tile([1, 32], F32, tag="invf")
        nc.gpsimd.dma_start(invf_sb[:], invf_dram[:])
        masks_sb = pconst.tile([128, 4 * 512], F32R, tag="masks")
        nc.gpsimd.dma_start(masks_sb[:], masks_dram[:].bitcast(F32R))
        lnw_sb = pconst.tile([128, LORA // 128], F32, tag="lnw")
        nc.gpsimd.dma_start(lnw_sb[:], lnw[:])
        pos_i = pconst.tile([1, TSLICE], I32, tag="posi")
        nc.gpsimd.dma_start(pos_i[:], pos[:])
        eps_sb = pconst.tile([128, 1], F32, tag="eps")
        nc.gpsimd.memset(eps_sb[:], EPS)

        # weights to SBUF
        wq_sb = pw.tile([128, KC * 256], BF16, tag="wq")
        for kc in range(KC):
            nc.gpsimd.dma_start(wq_sb[:, kc * 256:(kc + 1) * 256],
                              wq_sh[kc * 128:(kc + 1) * 128, :])
        wkvb_sb = pw.tile([128, LC * 512], BF16, tag="wkvb")
        for lc in range(LC):
            nc.gpsimd.dma_start(wkvb_sb[:, lc * 512:(lc + 1) * 512],
                              wkv_b_sh[lc * 128:(lc + 1) * 128, :])
        wo_sb = pw.tile([128, HPC * H], F32R, tag="wo")
        for hh in range(HPC):
            nc.gpsimd.dma_start(wo_sb[:, hh * H:(hh + 1) * H],
                              wo_sh[hh * 128:(hh + 1) * 128, :].bitcast(F32R))

        # DRAM bounce buffers for the latent all-gather
        cc_in = pdram.tile([TSLICE, LORA], BF16, tag="ccin")
        cc_out = pdram.tile([NTOK, LORA], BF16, tag="ccout",
                            addr_space="Shared")

        # =============== stage 0 + q-projection (streaming pools) ========
        with tc.tile_pool(name="pstream", bufs=2) as pstream, \
             tc.tile_pool(name="pwkva", bufs=1) as pwkva, \
             tc.tile_pool(name="ps0w", bufs=1) as ps0w, \
             tc.tile_pool(name="psA", bufs=1, space="PSUM") as psA, \
             tc.tile_pool(name="psB", bufs=1, space="PSUM") as psB:

            wkva_sb = pwkva.tile([128, KC * (LORA + D_ROPE)], F32R, tag="wkva")
            for kc in range(KC):
                nc.gpsimd.dma_start(
                    wkva_sb[:, kc * 576:(kc + 1) * 576],
                    wkv_a[kc * 128:(kc + 1) * 128, :].bitcast(F32R))

            # --- rope angle table (freq on partitions, token on free) ---
            pos_f = ps0w.tile([1, TSLICE], F32, tag="posf")
            nc.vector.tensor_copy(pos_f[:], pos_i[:])
            ang_ps = psA.tile([32, TSLICE], F32, tag="ang")
            nc.tensor.matmul(ang_ps[:], invf_sb[:], pos_f[:])
            ang = ps0w.tile([32, TSLICE], F32, tag="ang_sb")
            nc.vector.tensor_copy(ang[:], ang_ps[:])
            kf = ps0w.tile([32, TSLICE], F32, tag="kf")
            ki = ps0w.tile([32, TSLICE], I32, tag="ki")
            nc.vector.tensor_scalar(kf[:], ang[:], 1.0 / TWO_PI, 0.5,
                                    mybir.AluOpType.mult, mybir.AluOpType.add)
            nc.vector.tensor_copy(ki[:], kf[:])      # trunc toward 0 (x>=0)
            nc.vector.tensor_copy(kf[:], ki[:])
            red = ps0w.tile([32, TSLICE], F32, tag="red")
            nc.vector.cody_waite_cascade(red[:], ang[:], kf[:], c1, c2, c3)
            red_s = ps0w.tile([32, TSLICE], F32, tag="reds")
            nc.vector.add_range_wrap(red_s[:], red[:], 0.0, PI, TWO_PI)
            red_c = ps0w.tile([32, TSLICE], F32, tag="redc")
            nc.vector.add_range_wrap(red_c[:], red[:], PI / 2.0, PI, TWO_PI)
            sin_sb = ps0w.tile([32, TSLICE], F32, tag="sin")
            nc.scalar.activation(sin_sb[:], red_s[:], AF.Sin)
            cos_sb = ps0w.tile([32, TSLICE], F32, tag="cos")
            nc.scalar.activation(cos_sb[:], red_c[:], AF.Sin)

            # --- ckvT = wkv_a^T @ hs_slice, m-inner so hs loads once ---
            ckv_ps = [psA.tile([128, TSLICE], F32, tag=f"ckv{m}",
                               name=f"ckv_ps{m}") for m in range(LC)]
            rope_ps = psA.tile([64, TSLICE], F32, tag="rope")
            for g in range(4):
                hs_g = pstream.tile([128, 4 * 512], F32R, tag="hsg0",
                                    name="hs_g")
                for i in range(4):
                    kc = g * 4 + i
                    nc.sync.dma_start(hs_g[:, i * 512:(i + 1) * 512],
                                      hslT[kc * 128:(kc + 1) * 128, :].bitcast(F32R))
                for i in range(4):
                    kc = g * 4 + i
                    rhs = (hs_g[:, i * 512:(i + 1) * 512])
                    st = (kc == 0)
                    sp = (kc == KC - 1)
                    for m in range(LC):
                        nc.tensor.matmul(
                            ckv_ps[m][:],
                            (wkva_sb[:, kc * 576 + m * 128: kc * 576 + (m + 1) * 128]),
                            rhs, start=st, stop=sp)
                    nc.tensor.matmul(
                        rope_ps[:],
                        (wkva_sb[:, kc * 576 + 512: kc * 576 + 576]),
                        rhs, start=st, stop=sp)

            # --- rmsnorm across partitions via ones-matmul broadcast ---
            ckv_raw = ps0w.tile([128, LC * TSLICE], F32, tag="ckvraw")
            sq = pstream.tile([128, TSLICE], F32R, tag="sq")
            ss_ps = psA.tile([128, TSLICE], F32, tag="ang")  # reuse ang slot
            for m in range(LC):
                nc.scalar.copy(ckv_raw[:, m * TSLICE:(m + 1) * TSLICE],
                               ckv_ps[m][:])
            for m in range(LC):
                sq_m = pstream.tile([128, TSLICE], F32R, tag="sq")
                nc.scalar.activation(
                    sq_m[:], ckv_raw[:, m * TSLICE:(m + 1) * TSLICE], AF.Square)
                nc.tensor.matmul(ss_ps[:], ones_sb[:], sq_m[:],
                                 start=(m == 0), stop=(m == LC - 1))
            rstd = ps0w.tile([128, TSLICE], F32, tag="rstd")
            nc.scalar.activation(rstd[:], ss_ps[:], AF.Sqrt,
                                 scale=1.0 / float(LORA), bias=eps_sb[:])
            nc.vector.reciprocal(rstd[:], rstd[:])
            cn = ps0w.tile([128, LC * TSLICE], F32, tag="cn")
            for m in range(LC):
                t = pstream.tile([128, TSLICE], F32, tag="cnt")
                nc.vector.tensor_scalar_mul(
                    t[:], ckv_raw[:, m * TSLICE:(m + 1) * TSLICE],
                    lnw_sb[:, m:m + 1])
                nc.vector.tensor_mul(cn[:, m * TSLICE:(m + 1) * TSLICE],
                                     t[:], rstd[:])
                nc.gpsimd.dma_start(kvfT[m * 128:(m + 1) * 128, :],
                                  cn[:, m * TSLICE:(m + 1) * TSLICE])
                cnb = pstream.tile([128, TSLICE], BF16, tag="cnb", name="cnb")
                nc.vector.tensor_copy(cnb[:],
                                      cn[:, m * TSLICE:(m + 1) * TSLICE])
                nc.gpsimd.dma_start(cc_in[m * 128:(m + 1) * 128, :], cnb[:])

            # --- rope combine & emit ---
            x1 = rope_ps[0:32, :]
            x2 = rope_ps[32:64, :]
            t1 = ps0w.tile([32, TSLICE], F32, tag="t1")
            t2 = ps0w.tile([32, TSLICE], F32, tag="t2")
            r1 = ps0w.tile([32, TSLICE], F32, tag="r1")
            r2 = ps0w.tile([32, TSLICE], F32, tag="r2")
            nc.vector.tensor_mul(t1[:], x1, cos_sb[:])
            nc.vector.tensor_mul(t2[:], x2, sin_sb[:])
            nc.vector.tensor_sub(r1[:], t1[:], t2[:])
            nc.gpsimd.dma_start(kvfT[512:544, :], r1[:])
            nc.vector.tensor_mul(t1[:], x2, cos_sb[:])
            nc.vector.tensor_mul(t2[:], x1, sin_sb[:])
            nc.vector.tensor_add(r2[:], t1[:], t2[:])
            nc.gpsimd.dma_start(kvfT[544:576, :], r2[:])

            # --- all-gather the normalized latent ---
            nc.gpsimd.collective_compute(
                "AllGather", mybir.AluOpType.bypass,
                replica_groups=[list(range(N_CORES))],
                ins=[cc_in[:].opt()], outs=[cc_out[:].opt()])

        # =============== per-sequence: decompress, attention, output ======
        with tc.tile_pool(name="pkv", bufs=2) as pkv, \
             tc.tile_pool(name="pmain", bufs=2) as pmain, \
             tc.tile_pool(name="pe", bufs=4) as pe, \
             tc.tile_pool(name="psC", bufs=1, space="PSUM") as psC, \
             tc.tile_pool(name="psD", bufs=2, space="PSUM") as psD:

            for s in range(B):
                toff = s * S
                # ---- q projection for this sequence, both heads ----
                qT_sb = pkv.tile([128, HPC * S], F32R, tag="qT",
                                 name="qT_sb")
                for tt in range(SEQ_TT):
                    for h in range(HPC):
                        psq = psD.tile([128, 512], F32, tag="scratch",
                                       name="psq")
                        for g in range(2):
                            if h == 0:
                                hs_g = pmain.tile([128, 8 * 512], BF16,
                                                  tag="hsg", name="hs_g",
                                                  bufs=3)
                                for i in range(8):
                                    kc = g * 8 + i
                                    nc.sync.dma_start(
                                        hs_g[:, i * 512:(i + 1) * 512],
                                        hsT[kc * 128:(kc + 1) * 128,
                                            toff + tt * 512:
                                            toff + (tt + 1) * 512])
                                if g == 0:
                                    hs_pair = [hs_g]
                                else:
                                    hs_pair.append(hs_g)
                            hs_g = hs_pair[g]
                            for i in range(8):
                                kc = g * 8 + i
                                nc.tensor.matmul(
                                    psq[:],
                                    (wq_sb[:, kc * 256 + h * 128:
                                           kc * 256 + (h + 1) * 128]),
                                    (hs_g[:, i * 512:(i + 1) * 512]),
                                    start=(kc == 0), stop=(kc == KC - 1))
                        nc.vector.tensor_copy(
                            qT_sb[:, h * S + tt * 512: h * S + (tt + 1) * 512],
                            psq[:])
                # ---- decompress: knopeT [d, tok] and v [tok, d] ----
                knT_sb = pkv.tile([128, HPC * S], F32R, tag="knT")
                v_sb = pkv.tile([128, SEQ_KT * 256], F32R, tag="v")
                for tt in range(SEQ_TT):
                    blk = (toff + tt * 512) // TSLICE
                    cc_g = pmain.tile([128, LC * 512], BF16, tag="ccg")
                    for lc in range(LC):
                        nc.sync.dma_start(
                            cc_g[:, lc * 512:(lc + 1) * 512],
                            cc_out[blk * TSLICE + lc * 128:
                                   blk * TSLICE + (lc + 1) * 128, :])
                    for h in range(HPC):
                        psk = psD.tile([128, 512], F32, tag="scratch", name="psk")
                        for lc in range(LC):
                            nc.tensor.matmul(
                                psk[:],
                                (wkvb_sb[:, lc * 512 + h * 128:
                                           lc * 512 + (h + 1) * 128]),
                                (cc_g[:, lc * 512:(lc + 1) * 512]),
                                start=(lc == 0), stop=(lc == LC - 1))
                        nc.vector.tensor_copy(
                            knT_sb[:, h * S + tt * 512: h * S + (tt + 1) * 512],
                            psk[:])
                    for mt in range(4):
                        psv = psD.tile([128, 256], F32, tag="scratch", name="psv")
                        for lc in range(LC):
                            nc.tensor.matmul(
                                psv[:],
                                (cc_g[:, lc * 512 + mt * 128:
                                        lc * 512 + (mt + 1) * 128]),
                                (wkvb_sb[:, lc * 512 + 256: lc * 512 + 512]),
                                start=(lc == 0), stop=(lc == LC - 1))
                        kt = tt * 4 + mt
                        nc.vector.tensor_copy(v_sb[:, kt * 256:(kt + 1) * 256], psv[:])

                # ---- attention + output projection per 512-query tile ----
                def outproj(qt, attn_sb):
                    for mt in range(4):
                        for ot in range(4):
                            pso = psD.tile([128, 512], F32, tag="scratch",
                                           name="pso")
                            for h in range(HPC):
                                nc.tensor.matmul(
                                    pso[:],
                                    (attn_sb[:, h * 512 + mt * 128:
                                               h * 512 + (mt + 1) * 128]),
                                    (wo_sb[:, h * H + ot * 512:
                                             h * H + (ot + 1) * 512]),
                                    start=(h == 0), stop=(h == HPC - 1))
                            osb = pmain.tile([128, 512], F32, tag="osb",
                                             name="osb")
                            nc.vector.tensor_copy(osb[:], pso[:])
                            nc.sync.dma_start(
                                out_p[toff + qt * 512 + mt * 128:
                                      toff + qt * 512 + (mt + 1) * 128,
                                      ot * 512:(ot + 1) * 512],
                                osb[:])

                prev = None
                for qt in range(SEQ_TT):
                    if prev is not None:
                        outproj(*prev)
                        prev = None
                    attn_sb = pmain.tile([128, HPC * 512], F32R, tag="attn")
                    nkt = 4 * qt + 4
                    psa = [psC.tile([128, 512], F32, tag=f"att{h}",
                                    name=f"psa{h}") for h in range(HPC)]
                    psd = [psC.tile([128, 512], F32, tag=f"den{h}",
                                    name=f"psd{h}") for h in range(HPC)]
                    for kt in range(nkt):
                        j = kt - 4 * qt
                        q0 = 128 * j if j > 0 else 0
                        for h in range(HPC):
                            pss = psD.tile([128, 512], F32, tag="sc",
                                           name="pss", bufs=2)
                            nc.tensor.matmul(
                                pss[:, q0:512],
                                (knT_sb[:, h * S + kt * 128:
                                          h * S + (kt + 1) * 128]),
                                (qT_sb[:, h * S + qt * 512 + q0:
                                         h * S + (qt + 1) * 512]),
                                start=True, stop=True)
                            e = pe.tile([128, 512], F32R, tag="e")
                            nc.scalar.activation(e[:, q0:512], pss[:, q0:512],
                                                 AF.Exp, scale=SCALING)
                            if j >= 0:
                                em = pe.tile([128, 512], F32R, tag="em")
                                nc.vector.tensor_mul(
                                    em[:, q0:512], e[:, q0:512],
                                    masks_sb[:, j * 512 + q0:(j + 1) * 512])
                                e_use = em
                            else:
                                e_use = e
                            nc.tensor.matmul(psd[h][:, q0:512], (ones_sb[:]),
                                             (e_use[:, q0:512]),
                                             start=(kt == 0),
                                             stop=(kt == nkt - 1))
                            nc.tensor.matmul(
                                psa[h][:, q0:512],
                                (v_sb[:, kt * 256 + h * 128:
                                        kt * 256 + (h + 1) * 128]),
                                (e_use[:, q0:512]),
                                start=(kt == 0), stop=(kt == nkt - 1))
                    for h in range(HPC):
                        rd = pmain.tile([128, 512], F32, tag="rd",
                                        name="rd")
                        rds = pmain.tile([128, 512], F32, tag="rds",
                                         name="rds")
                        nc.vector.reciprocal_approx_accurate(rd[:], psd[h][:],
                                                             rds[:])
                        nc.vector.tensor_mul(
                            attn_sb[:, h * 512:(h + 1) * 512], psa[h][:],
                            rd[:])
                    prev = (qt, attn_sb)
                outproj(*prev)

        for p in reversed(ctx_pools):
            p.release()

    nc.compile()
    return nc


LC_HOST = LORA // 128

_NC_CACHE = None


def _get_nc():
    global _NC_CACHE
    if _NC_CACHE is None:
        _NC_CACHE = _build()
    return _NC_CACHE


def _make_in_maps(positions, hidden_states, wq, wkv_a, ln_w, wkv_b, wo):
    positions = np.asarray(positions, np.int32)
    hidden_states = np.asarray(hidden_states, np.float32)
    wq = np.asarray(wq, np.float32)
    wkv_a = np.asarray(wkv_a, np.float32)
    ln_w = np.asarray(ln_w, np.float32)
    wkv_b = np.asarray(wkv_b, np.float32)
    wo = np.asarray(wo, np.float32)

    hs_flat = hidden_states.reshape(NTOK, H)
    hsT = np.ascontiguousarray(hs_flat.T)
    hsT_bf = hsT.astype(ml_dtypes.bfloat16)
    pos_flat = positions.reshape(NTOK)
    lnw_tile = np.ascontiguousarray(ln_w.reshape(LC_HOST, 128).T)

    in_maps = []
    for c in range(N_CORES):
        heads = [HPC * c + j for j in range(HPC)]
        wq_sh = np.ascontiguousarray(np.concatenate(
            [wq[:, h * 192: h * 192 + 128] for h in heads],
            axis=1)).astype(ml_dtypes.bfloat16)
        wkv_b_sh = np.ascontiguousarray(np.concatenate(
            [wkv_b[:, h * 256: h * 256 + 128] for h in heads] +
            [wkv_b[:, h * 256 + 128: h * 256 + 256] for h in heads],
            axis=1)).astype(ml_dtypes.bfloat16)
        wo_sh = np.ascontiguousarray(np.concatenate(
            [wo[h * 128:(h + 1) * 128, :] for h in heads], axis=0))
        in_maps.append({
            "hsT": hsT_bf,
            "hslT": np.ascontiguousarray(hsT[:, c * TSLICE:(c + 1) * TSLICE]),
            "pos": np.ascontiguousarray(
                pos_flat[c * TSLICE:(c + 1) * TSLICE].reshape(1, TSLICE)),
            "wq_sh": wq_sh,
            "wkv_a": wkv_a,
            "lnw": lnw_tile,
            "wkv_b_sh": wkv_b_sh,
            "wo_sh": wo_sh,
        })
    return in_maps


def kernel(**inputs):
    nc = _get_nc()
    in_maps = _make_in_maps(**inputs)
    res = run_bass_kernel_spmd(nc, in_maps, list(range(N_CORES)))
    out = np.zeros((NTOK, H), np.float32)
    for c in range(N_CORES):
        out += res.results[c]["out_p"]
    kvf = np.concatenate(
        [res.results[c]["kvfT"].T for c in range(N_CORES)], axis=0)
    return out.reshape(B, S, H), np.ascontiguousarray(kvf.reshape(B, S, 576))


def run_profiled(**inputs):
    nc = _get_nc()
    in_maps = _make_in_maps(**inputs)
    return run_bass_kernel_spmd(nc, in_maps, list(range(N_CORES)), trace=True)


# revision 27
# speedup vs baseline: 1.0658x; 1.0658x over previous
"""DeepseekV2 (MLA) attention prefill kernel for 8 Trainium2 NeuronCores.

Sharding: tensor-parallel over the 16 heads (2 heads/core: wq/wkv_b output
cols + wo input rows sharded), token-sliced low-rank KV stage (each core
computes 512 tokens of the compressed latent, AllGather redistributes it),
host sums the 8 partial output projections.

Layout strategy (everything "transposed", feature-on-partition):
  - hsT [H, B*S] streamed from DRAM; q/k projections produce qT/knopeT
    [d, tok] directly, V is produced in [tok, d] — the exact operand
    layouts attention needs, so the kernel contains zero transposes.
  - scores are computed as scoresT [k_tok, q_tok]; softmax skips the max
    subtraction (|score*scale| <= ~8 for this model, exp is safe), the
    denominator comes from a ones-matmul that also broadcasts it across
    partitions, so normalization is a plain elementwise multiply.
"""

import sys

if "/opt/trn_rl_repo" not in sys.path:
    sys.path.insert(0, "/opt/trn_rl_repo")

import numpy as np
import ml_dtypes

import concourse.bass as bass
import concourse.mybir as mybir
import concourse.tile as tile
from concourse import bacc
from concourse import bass_isa
from concourse.bass_utils import run_bass_kernel_spmd

F32 = mybir.dt.float32
F32R = mybir.dt.float32r
BF16 = mybir.dt.bfloat16
I32 = mybir.dt.int32
AF = mybir.ActivationFunctionType

B, S, H = 2, 2048, 2048
NH = 16
D_NOPE = 128
D_ROPE = 64
D_V = 128
LORA = 512
ROPE_BASE = 1e6
SCALING = float(D_V) ** -0.5
EPS = 1e-6

N_CORES = 8
NTOK = B * S                 # 4096 flattened tokens
TSLICE = NTOK // N_CORES     # 512 tokens/core for the latent stage
HPC = NH // N_CORES          # 2 heads per core

PI = float(np.pi)
TWO_PI = 2.0 * PI


def _cody_waite_consts():
    """Split 2*pi into c1+c2+c3 with c1/c2 exact under k<=~512 multiply."""
    def trunc12(x):
        u = np.float32(x).view(np.uint32) & np.uint32(0xFFFFF000)
        return float(u.view(np.float32))
    c1 = trunc12(TWO_PI)
    c2 = trunc12(TWO_PI - c1)
    c3 = float(np.float32(TWO_PI - c1 - c2))
    return c1, c2, c3


def _build():
    nc = bacc.Bacc("TRN2", target_bir_lowering=False, debug=False,
                   num_devices=N_CORES)

    # ---- per-core I/O ----
    hsT = nc.declare_dram_parameter("hsT", [H, NTOK], BF16, isOutput=False)
    hslT = nc.declare_dram_parameter("hslT", [H, TSLICE], F32, isOutput=False)
    pos = nc.declare_dram_parameter("pos", [1, TSLICE], I32, isOutput=False)
    wq_sh = nc.declare_dram_parameter("wq_sh", [H, HPC * D_NOPE], BF16, isOutput=False)
    wkv_a = nc.declare_dram_parameter("wkv_a", [H, LORA + D_ROPE], F32, isOutput=False)
    lnw = nc.declare_dram_parameter("lnw", [128, LORA // 128], F32, isOutput=False)
    # [LORA, 512]: cols = [h0_k(128) | h1_k(128) | h0_v(128) | h1_v(128)]
    wkv_b_sh = nc.declare_dram_parameter("wkv_b_sh", [LORA, HPC * (D_NOPE + D_V)],
                                         BF16, isOutput=False)
    wo_sh = nc.declare_dram_parameter("wo_sh", [HPC * D_V, H], F32, isOutput=False)

    out_p = nc.declare_dram_parameter("out_p", [NTOK, H], F32, isOutput=True)
    kvfT = nc.declare_dram_parameter("kvfT", [LORA + D_ROPE, TSLICE], F32,
                                     isOutput=True)

    # ---- constants (baked into the NEFF) ----
    ones_np = np.ones((128, 128), np.float32)
    invf_np = (1.0 / (ROPE_BASE ** (np.arange(32, dtype=np.float64) / 32.0))
               ).astype(np.float32).reshape(1, 32)
    # mask_j[p, f] = 1 if f >= p + 128*j (valid, k<=q) else 0 ; j = 0..3
    f_idx = np.arange(512)[None, :]
    p_idx = np.arange(128)[:, None]
    masks_np = np.concatenate(
        [(f_idx >= p_idx + 128 * j).astype(np.float32) for j in range(4)], axis=1)
    ones_dram = nc.inline_tensor(ones_np, name="ones_c")
    invf_dram = nc.inline_tensor(invf_np, name="invf_c")
    masks_dram = nc.inline_tensor(masks_np, name="masks_c")

    c1, c2, c3 = _cody_waite_consts()

    KC = H // 128        # 16 contraction chunks over hidden dim
    LC = LORA // 128     # 4 contraction chunks over lora dim
    SEQ_TT = S // 512    # 4 512-token tiles per sequence
    SEQ_KT = S // 128    # 16 128-token key tiles per sequence

    with tile.TileContext(nc) as tc:
        ctx_pools = []

        pconst = tc.alloc_tile_pool(name="pconst", bufs=1)
        pw = tc.alloc_tile_pool(name="pw", bufs=1)
        pqt = tc.alloc_tile_pool(name="pqt", bufs=1)
        pdram = tc.alloc_tile_pool(name="pdram", bufs=1, space="DRAM")
        ctx_pools += [pconst, pw, pqt, pdram]

        # constants to SBUF
        ones_sb = pconst.tile([128, 128], F32R, tag="ones")
        nc.gpsimd.dma_start(ones_sb[:], ones_dram[:].bitcast(F32R))
        invf_sb = pconst.tile([1, 32], F32, tag="invf")
        nc.gpsimd.dma_start(invf_sb[:], invf_dram[:])
        masks_sb = pconst.tile([128, 4 * 512], F32R, tag="masks")
        nc.gpsimd.dma_start(masks_sb[:], masks_dram[:].bitcast(F32R))
        lnw_sb = pconst.tile([128, LORA // 128], F32, tag="lnw")
        nc.gpsimd.dma_start(lnw_sb[:], lnw[:])
        pos_i = pconst.tile([1, TSLICE], I32, tag="posi")
        nc.gpsimd.dma_start(pos_i[:], pos[:])
        eps_sb = pconst.tile([128, 1], F32, tag="eps")
        nc.gpsimd.memset(eps_sb[:], EPS)

        # weights to SBUF
        wq_sb = pw.tile([128, KC * 256], BF16, tag="wq")
        for kc in range(KC):
            nc.gpsimd.dma_start(wq_sb[:, kc * 256:(kc + 1) * 256],
                              wq_sh[kc * 128:(kc + 1) * 128, :])
        wkvb_sb = pw.tile([128, LC * 512], BF16, tag="wkvb")
        for lc in range(LC):
            nc.gpsimd.dma_start(wkvb_sb[:, lc * 512:(lc + 1) * 512],
                              wkv_b_sh[lc * 128:(lc + 1) * 128, :])
        wo_sb = pw.tile([128, HPC * H], F32R, tag="wo")
        for hh in range(HPC):
            nc.gpsimd.dma_start(wo_sb[:, hh * H:(hh + 1) * H],
                              wo_sh[hh * 128:(hh + 1) * 128, :].bitcast(F32R))

        # qT for both seqs & both heads
        qT_sb = pqt.tile([128, HPC * NTOK], F32R, tag="qT")

        # DRAM bounce buffers for the latent all-gather
        cc_in = pdram.tile([TSLICE, LORA], BF16, tag="ccin")
        cc_out = pdram.tile([NTOK, LORA], BF16, tag="ccout",
                            addr_space="Shared")

        # =============== stage 0 + q-projection (streaming pools) ========
        with tc.tile_pool(name="pstream", bufs=2) as pstream, \
             tc.tile_pool(name="pwkva", bufs=1) as pwkva, \
             tc.tile_pool(name="ps0w", bufs=1) as ps0w, \
             tc.tile_pool(name="psA", bufs=1, space="PSUM") as psA, \
             tc.tile_pool(name="psB", bufs=1, space="PSUM") as psB:

            wkva_sb = pwkva.tile([128, KC * (LORA + D_ROPE)], F32R, tag="wkva")
            for kc in range(KC):
                nc.gpsimd.dma_start(
                    wkva_sb[:, kc * 576:(kc + 1) * 576],
                    wkv_a[kc * 128:(kc + 1) * 128, :].bitcast(F32R))

            # --- rope angle table (freq on partitions, token on free) ---
            pos_f = ps0w.tile([1, TSLICE], F32, tag="posf")
            nc.vector.tensor_copy(pos_f[:], pos_i[:])
            ang_ps = psA.tile([32, TSLICE], F32, tag="ang")
            nc.tensor.matmul(ang_ps[:], invf_sb[:], pos_f[:])
            ang = ps0w.tile([32, TSLICE], F32, tag="ang_sb")
            nc.vector.tensor_copy(ang[:], ang_ps[:])
            kf = ps0w.tile([32, TSLICE], F32, tag="kf")
            ki = ps0w.tile([32, TSLICE], I32, tag="ki")
            nc.vector.tensor_scalar(kf[:], ang[:], 1.0 / TWO_PI, 0.5,
                                    mybir.AluOpType.mult, mybir.AluOpType.add)
            nc.vector.tensor_copy(ki[:], kf[:])      # trunc toward 0 (x>=0)
            nc.vector.tensor_copy(kf[:], ki[:])
            red = ps0w.tile([32, TSLICE], F32, tag="red")
            nc.vector.cody_waite_cascade(red[:], ang[:], kf[:], c1, c2, c3)
            red_s = ps0w.tile([32, TSLICE], F32, tag="reds")
            nc.vector.add_range_wrap(red_s[:], red[:], 0.0, PI, TWO_PI)
            red_c = ps0w.tile([32, TSLICE], F32, tag="redc")
            nc.vector.add_range_wrap(red_c[:], red[:], PI / 2.0, PI, TWO_PI)
            sin_sb = ps0w.tile([32, TSLICE], F32, tag="sin")
            nc.scalar.activation(sin_sb[:], red_s[:], AF.Sin)
            cos_sb = ps0w.tile([32, TSLICE], F32, tag="cos")
            nc.scalar.activation(cos_sb[:], red_c[:], AF.Sin)

            # --- ckvT = wkv_a^T @ hs_slice, m-inner so hs loads once ---
            ckv_ps = [psA.tile([128, TSLICE], F32, tag=f"ckv{m}",
                               name=f"ckv_ps{m}") for m in range(LC)]
            rope_ps = psA.tile([64, TSLICE], F32, tag="rope")
            for g in range(4):
                hs_g = pstream.tile([128, 4 * 512], F32R, tag="hsg0",
                                    name="hs_g")
                for i in range(4):
                    kc = g * 4 + i
                    nc.sync.dma_start(hs_g[:, i * 512:(i + 1) * 512],
                                      hslT[kc * 128:(kc + 1) * 128, :].bitcast(F32R))
                for i in range(4):
                    kc = g * 4 + i
                    rhs = (hs_g[:, i * 512:(i + 1) * 512])
                    st = (kc == 0)
                    sp = (kc == KC - 1)
                    for m in range(LC):
                        nc.tensor.matmul(
                            ckv_ps[m][:],
                            (wkva_sb[:, kc * 576 + m * 128: kc * 576 + (m + 1) * 128]),
                            rhs, start=st, stop=sp)
                    nc.tensor.matmul(
                        rope_ps[:],
                        (wkva_sb[:, kc * 576 + 512: kc * 576 + 576]),
                        rhs, start=st, stop=sp)

            # --- rmsnorm across partitions via ones-matmul broadcast ---
            ckv_raw = ps0w.tile([128, LC * TSLICE], F32, tag="ckvraw")
            sq = pstream.tile([128, TSLICE], F32R, tag="sq")
            ss_ps = psA.tile([128, TSLICE], F32, tag="ang")  # reuse ang slot
            for m in range(LC):
                nc.scalar.copy(ckv_raw[:, m * TSLICE:(m + 1) * TSLICE],
                               ckv_ps[m][:])
            for m in range(LC):
                sq_m = pstream.tile([128, TSLICE], F32R, tag="sq")
                nc.scalar.activation(
                    sq_m[:], ckv_raw[:, m * TSLICE:(m + 1) * TSLICE], AF.Square)
                nc.tensor.matmul(ss_ps[:], ones_sb[:], sq_m[:],
                                 start=(m == 0), stop=(m == LC - 1))
            rstd = ps0w.tile([128, TSLICE], F32, tag="rstd")
            nc.scalar.activation(rstd[:], ss_ps[:], AF.Sqrt,
                                 scale=1.0 / float(LORA), bias=eps_sb[:])
            nc.vector.reciprocal(rstd[:], rstd[:])
            cn = ps0w.tile([128, LC * TSLICE], F32, tag="cn")
            for m in range(LC):
                t = pstream.tile([128, TSLICE], F32, tag="cnt")
                nc.vector.tensor_scalar_mul(
                    t[:], ckv_raw[:, m * TSLICE:(m + 1) * TSLICE],
                    lnw_sb[:, m:m + 1])
                nc.vector.tensor_mul(cn[:, m * TSLICE:(m + 1) * TSLICE],
                                     t[:], rstd[:])
                nc.gpsimd.dma_start(kvfT[m * 128:(m + 1) * 128, :],
                                  cn[:, m * TSLICE:(m + 1) * TSLICE])
                cnb = pstream.tile([128, TSLICE], BF16, tag="cnb", name="cnb")
                nc.vector.tensor_copy(cnb[:],
                                      cn[:, m * TSLICE:(m + 1) * TSLICE])
                nc.gpsimd.dma_start(cc_in[m * 128:(m + 1) * 128, :], cnb[:])

            # --- rope combine & emit ---
            x1 = rope_ps[0:32, :]
            x2 = rope_ps[32:64, :]
            t1 = ps0w.tile([32, TSLICE], F32, tag="t1")
            t2 = ps0w.tile([32, TSLICE], F32, tag="t2")
            r1 = ps0w.tile([32, TSLICE], F32, tag="r1")
            r2 = ps0w.tile([32, TSLICE], F32, tag="r2")
            nc.vector.tensor_mul(t1[:], x1, cos_sb[:])
            nc.vector.tensor_mul(t2[:], x2, sin_sb[:])
            nc.vector.tensor_sub(r1[:], t1[:], t2[:])
            nc.gpsimd.dma_start(kvfT[512:544, :], r1[:])
            nc.vector.tensor_mul(t1[:], x2, cos_sb[:])
            nc.vector.tensor_mul(t2[:], x1, sin_sb[:])
            nc.vector.tensor_add(r2[:], t1[:], t2[:])
            nc.gpsimd.dma_start(kvfT[544:576, :], r2[:])

            # --- all-gather the normalized latent ---
            nc.gpsimd.collective_compute(
                "AllGather", mybir.AluOpType.bypass,
                replica_groups=[list(range(N_CORES))],
                ins=[cc_in[:].opt()], outs=[cc_out[:].opt()])

            # --- q projection for all tokens, both heads ---
            for tt in range(NTOK // 512):
                psq = [psB.tile([128, 512], F32, tag=f"proj{h}",
                                name=f"psq{h}") for h in range(HPC)]
                for g in range(2):
                    hs_g = pstream.tile([128, 8 * 512], BF16, tag="hsg",
                                        name="hs_g")
                    for i in range(8):
                        kc = g * 8 + i
                        nc.sync.dma_start(
                            hs_g[:, i * 512:(i + 1) * 512],
                            hsT[kc * 128:(kc + 1) * 128,
                                tt * 512:(tt + 1) * 512])
                    for h in range(HPC):
                        for i in range(8):
                            kc = g * 8 + i
                            nc.tensor.matmul(
                                psq[h][:],
                                (wq_sb[:, kc * 256 + h * 128:
                                       kc * 256 + (h + 1) * 128]),
                                (hs_g[:, i * 512:(i + 1) * 512]),
                                start=(kc == 0), stop=(kc == KC - 1))
                for h in range(HPC):
                    nc.vector.tensor_copy(
                        qT_sb[:, h * NTOK + tt * 512: h * NTOK + (tt + 1) * 512],
                        psq[h][:])

        # =============== per-sequence: decompress, attention, output ======
        with tc.tile_pool(name="pkv", bufs=2) as pkv, \
             tc.tile_pool(name="pmain", bufs=2) as pmain, \
             tc.tile_pool(name="pe", bufs=4) as pe, \
             tc.tile_pool(name="psC", bufs=1, space="PSUM") as psC, \
             tc.tile_pool(name="psD", bufs=2, space="PSUM") as psD:

            for s in range(B):
                toff = s * S
                # ---- decompress: knopeT [d, tok] and v [tok, d] ----
                knT_sb = pkv.tile([128, HPC * S], F32R, tag="knT")
                v_sb = pkv.tile([128, SEQ_KT * 256], F32R, tag="v")
                for tt in range(SEQ_TT):
                    blk = (toff + tt * 512) // TSLICE
                    cc_g = pmain.tile([128, LC * 512], BF16, tag="ccg")
                    for lc in range(LC):
                        nc.sync.dma_start(
                            cc_g[:, lc * 512:(lc + 1) * 512],
                            cc_out[blk * TSLICE + lc * 128:
                                   blk * TSLICE + (lc + 1) * 128, :])
                    for h in range(HPC):
                        psk = psD.tile([128, 512], F32, tag="scratch", name="psk")
                        for lc in range(LC):
                            nc.tensor.matmul(
                                psk[:],
                                (wkvb_sb[:, lc * 512 + h * 128:
                                           lc * 512 + (h + 1) * 128]),
                                (cc_g[:, lc * 512:(lc + 1) * 512]),
                                start=(lc == 0), stop=(lc == LC - 1))
                        nc.vector.tensor_copy(
                            knT_sb[:, h * S + tt * 512: h * S + (tt + 1) * 512],
                            psk[:])
                    for mt in range(4):
                        psv = psD.tile([128, 256], F32, tag="scratch", name="psv")
                        for lc in range(LC):
                            nc.tensor.matmul(
                                psv[:],
                                (cc_g[:, lc * 512 + mt * 128:
                                        lc * 512 + (mt + 1) * 128]),
                                (wkvb_sb[:, lc * 512 + 256: lc * 512 + 512]),
                                start=(lc == 0), stop=(lc == LC - 1))
                        kt = tt * 4 + mt
                        nc.vector.tensor_copy(v_sb[:, kt * 256:(kt + 1) * 256], psv[:])

                # ---- attention + output projection per 512-query tile ----
                def outproj(qt, attn_sb):
                    for mt in range(4):
                        for ot in range(4):
                            pso = psD.tile([128, 512], F32, tag="scratch",
                                           name="pso")
                            for h in range(HPC):
                                nc.tensor.matmul(
                                    pso[:],
                                    (attn_sb[:, h * 512 + mt * 128:
                                               h * 512 + (mt + 1) * 128]),
                                    (wo_sb[:, h * H + ot * 512:
                                             h * H + (ot + 1) * 512]),
                                    start=(h == 0), stop=(h == HPC - 1))
                            osb = pmain.tile([128, 512], F32, tag="osb",
                                             name="osb")
                            nc.vector.tensor_copy(osb[:], pso[:])
                            nc.sync.dma_start(
                                out_p[toff + qt * 512 + mt * 128:
                                      toff + qt * 512 + (mt + 1) * 128,
                                      ot * 512:(ot + 1) * 512],
                                osb[:])

                prev = None
                for qt in range(SEQ_TT):
                    if prev is not None:
                        outproj(*prev)
                        prev = None
                    attn_sb = pmain.tile([128, HPC * 512], F32R, tag="attn")
                    nkt = 4 * qt + 4
                    psa = [psC.tile([128, 512], F32, tag=f"att{h}",
                                    name=f"psa{h}") for h in range(HPC)]
                    psd = [psC.tile([128, 512], F32, tag=f"den{h}",
                                    name=f"psd{h}") for h in range(HPC)]
                    for kt in range(nkt):
                        j = kt - 4 * qt
                        q0 = 128 * j if j > 0 else 0
                        for h in range(HPC):
                            pss = psD.tile([128, 512], F32, tag="sc",
                                           name="pss", bufs=2)
                            nc.tensor.matmul(
                                pss[:, q0:512],
                                (knT_sb[:, h * S + kt * 128:
                                          h * S + (kt + 1) * 128]),
                                (qT_sb[:, h * NTOK + toff + qt * 512 + q0:
                                         h * NTOK + toff + (qt + 1) * 512]),
                                start=True, stop=True)
                            e = pe.tile([128, 512], F32R, tag="e")
                            nc.scalar.activation(e[:, q0:512], pss[:, q0:512],
                                                 AF.Exp, scale=SCALING)
                            if j >= 0:
                                em = pe.tile([128, 512], F32R, tag="em")
                                nc.vector.tensor_mul(
                                    em[:, q0:512], e[:, q0:512],
                                    masks_sb[:, j * 512 + q0:(j + 1) * 512])
                                e_use = em
                            else:
                                e_use = e
                            nc.tensor.matmul(psd[h][:, q0:512], (ones_sb[:]),
                                             (e_use[:, q0:512]),
                                             start=(kt == 0),
                                             stop=(kt == nkt - 1))
                            nc.tensor.matmul(
                                psa[h][:, q0:512],
                                (v_sb[:, kt * 256 + h * 128:
                                        kt * 256 + (h + 1) * 128]),
                                (e_use[:, q0:512]),
                                start=(kt == 0), stop=(kt == nkt - 1))
                    for h in range(HPC):
                        rd = pmain.tile([128, 512], F32, tag="rd",
                                        name="rd")
                        rds = pmain.tile([128, 512], F32, tag="rds",
                                         name="rds")
                        nc.vector.reciprocal_approx_accurate(rd[:], psd[h][:],
                                                             rds[:])
                        nc.vector.tensor_mul(
                            attn_sb[:, h * 512:(h + 1) * 512], psa[h][:],
                            rd[:])
                    prev = (qt, attn_sb)
                outproj(*prev)

        for p in reversed(ctx_pools):
            p.release()

    nc.compile()
    return nc


LC_HOST = LORA // 128

_NC_CACHE = None


def _get_nc():
    global _NC_CACHE
    if _NC_CACHE is None:
        _NC_CACHE = _build()
    return _NC_CACHE


def _make_in_maps(positions, hidden_states, wq, wkv_a, ln_w, wkv_b, wo):
    positions = np.asarray(positions, np.int32)
    hidden_states = np.asarray(hidden_states, np.float32)
    wq = np.asarray(wq, np.float32)
    wkv_a = np.asarray(wkv_a, np.float32)
    ln_w = np.asarray(ln_w, np.float32)
    wkv_b = np.asarray(wkv_b, np.float32)
    wo = np.asarray(wo, np.float32)

    hs_flat = hidden_states.reshape(NTOK, H)
    hsT = np.ascontiguousarray(hs_flat.T)
    hsT_bf = hsT.astype(ml_dtypes.bfloat16)
    pos_flat = positions.reshape(NTOK)
    lnw_tile = np.ascontiguousarray(ln_w.reshape(LC_HOST, 128).T)

    in_maps = []
    for c in range(N_CORES):
        heads = [HPC * c + j for j in range(HPC)]
        wq_sh = np.ascontiguousarray(np.concatenate(
            [wq[:, h * 192: h * 192 + 128] for h in heads],
            axis=1)).astype(ml_dtypes.bfloat16)
        wkv_b_sh = np.ascontiguousarray(np.concatenate(
            [wkv_b[:, h * 256: h * 256 + 128] for h in heads] +
            [wkv_b[:, h * 256 + 128: h * 256 + 256] for h in heads],
            axis=1)).astype(ml_dtypes.bfloat16)
        wo_sh = np.ascontiguousarray(np.concatenate(
            [wo[h * 128:(h + 1) * 128, :] for h in heads], axis=0))
        in_maps.append({
            "hsT": hsT_bf,
            "hslT": np.ascontiguousarray(hsT[:, c * TSLICE:(c + 1) * TSLICE]),
            "pos": np.ascontiguousarray(
                pos_flat[c * TSLICE:(c + 1) * TSLICE].reshape(1, TSLICE)),
            "wq_sh": wq_sh,
            "wkv_a": wkv_a,
            "lnw": lnw_tile,
            "wkv_b_sh": wkv_b_sh,
            "wo_sh": wo_sh,
        })
    return in_maps


def kernel(**inputs):
    nc = _get_nc()
    in_maps = _make_in_maps(**inputs)
    res = run_bass_kernel_spmd(nc, in_maps, list(range(N_CORES)))
    out = np.zeros((NTOK, H), np.float32)
    for c in range(N_CORES):
        out += res.results[c]["out_p"]
    kvf = np.concatenate(
        [res.results[c]["kvfT"].T for c in range(N_CORES)], axis=0)
    return out.reshape(B, S, H), np.ascontiguousarray(kvf.reshape(B, S, 576))


def run_profiled(**inputs):
    nc = _get_nc()
    in_maps = _make_in_maps(**inputs)
    return run_bass_kernel_spmd(nc, in_maps, list(range(N_CORES)), trace=True)
